# revision 17
# baseline (speedup 1.0000x reference)
# Trainium2 Bass kernel for nn_Decoder (LSTM decoder + GCN message passing).
#
# Strategy (8 NeuronCores, SPMD):
#   * Data-parallel over nodes N=10000 -> 1250 nodes/core for fc2 + LSTM +
#     projection. State kept feature-major ([H, nodes]) so every matmul is
#     PE-friendly with K=H=128 and no transposes.
#   * Algebraic rewrite: the GCN aggregation and fc3 are both linear, so
#     aggregate AFTER projecting features to NF=16:
#        x_hat[n,t] = dinv[n] * sum_{e: dst=n} (dinv[src] * mask[src] * hs[t,src] @ (W_gcn@W_fc3))
#                     + (b_gcn@W_fc3 + b_fc3)
#     This shrinks the scatter/gather payload 8x (H=128 -> NF=16 per t).
#   * Y table ([N, T*NF], fp16, dinv*mask pre-scaled) is AllGather'ed across
#     the 8 cores. The scatter-add over edges is reformulated as a
#     block-sparse matmul: the host densifies the normalized adjacency into
#     128x128 blocks A[dst_tile, src_blk] (entry = edge multiplicity), and
#     each core computes agg[dst_tile] = sum_sb A[dst_tile,sb].T @ Y[sb]
#     streaming Y sequentially (direct DMA only - indirect DMA's per-row
#     descriptor generation on GpSimd would cost ~200us).
import os
import numpy as np

import concourse.bass as bass
import concourse.bacc as bacc
import concourse.tile as tile
from concourse import mybir
from concourse import bass_utils

P = 128
N, T, NF, H, L, E = 10000, 12, 16, 128, 64, 160000
NCORES = 8
NCN = N // NCORES            # 1250 nodes per core
NTILES = (NCN + P - 1) // P  # 10 dst tiles per core
NSB = (N + P - 1) // P       # 79 source blocks (last has 16 rows)
CH = [(0, 512), (512, 512), (1024, NCN - 1024)]  # LSTM node chunks (<=512)
TNF = T * NF                 # 192

F32 = mybir.dt.float32
F16 = mybir.dt.float16
U8 = mybir.dt.uint8

# gate q: 0=i, 1=f, 2=g, 3=o ; activation: sigmoid for i,f,o ; tanh for g
GATE_FUNCS = ["Sigmoid", "Sigmoid", "Tanh", "Sigmoid"]

_BUILD_CACHE = {}
LAST_RESULTS = None  # BassKernelResults of the most recent run (for test harness)


def _build():
    nc = bacc.Bacc("TRN2", target_bir_lowering=False, debug=False,
                   num_devices=NCORES)

    # ---------------- I/O declarations ----------------
    zT = nc.dram_tensor("zT", [L, NCN], F32, kind="ExternalInput")
    xm = nc.dram_tensor("xm", [NCN, TNF], U8, kind="ExternalInput")
    wfc2 = nc.dram_tensor("wfc2", [L, H], F32, kind="ExternalInput")
    b2 = nc.dram_tensor("b2", [P, 1], F32, kind="ExternalInput")
    wih = nc.dram_tensor("wih", [H, 4 * H], F16, kind="ExternalInput")
    whh = nc.dram_tensor("whh", [H, 4 * H], F16, kind="ExternalInput")
    bg = nc.dram_tensor("bg", [P, 4], F32, kind="ExternalInput")
    wcomb = nc.dram_tensor("wcomb", [H, NF], F16, kind="ExternalInput")
    bout = nc.dram_tensor("bout", [P, TNF], F32, kind="ExternalInput")
    dinvt = nc.dram_tensor("dinvt", [P, NTILES], F32, kind="ExternalInput")
    # A-blocks, wave-major: row (w*NSB + sb)*128 + p ; col = k_local*128 + drel
    ablk = nc.dram_tensor("ablk", [2 * NSB * P, 5 * P], F16,
                          kind="ExternalInput")
    xhat = nc.dram_tensor("xhat", [NCN, TNF], F32, kind="ExternalOutput")

    with tile.TileContext(nc) as tc:
        with tc.tile_pool(name="cpool", bufs=1) as cp, \
             tc.tile_pool(name="spool", bufs=1) as sp, \
             tc.tile_pool(name="dram", bufs=1, space="DRAM") as dp:

            # ---- constant loads ----
            zt_sb = cp.tile([L, NCN], F32)
            nc.sync.dma_start(zt_sb[:], zT[:])
            wfc2_sb = cp.tile([L, H], F32)
            nc.sync.dma_start(wfc2_sb[:], wfc2[:])
            b2_sb = cp.tile([P, 1], F32)
            nc.sync.dma_start(b2_sb[:], b2[:])
            wih_sb = cp.tile([H, 4 * H], F16)
            nc.sync.dma_start(wih_sb[:], wih[:])
            whh_sb = cp.tile([H, 4 * H], F16)
            nc.sync.dma_start(whh_sb[:], whh[:])
            bg_sb = cp.tile([P, 4], F32)
            nc.sync.dma_start(bg_sb[:], bg[:])
            wcomb_sb = cp.tile([H, NF], F16)
            nc.sync.dma_start(wcomb_sb[:], wcomb[:])
            bout_sb = cp.tile([P, TNF], F32)
            nc.sync.dma_start(bout_sb[:], bout[:])
            dinv_sb = cp.tile([P, NTILES], F32)
            nc.sync.dma_start(dinv_sb[:], dinvt[:])

            SL = [(0, 96), (96, 48), (144, 48)]  # (col0, width) per AG slice
            yshard_s = [dp.tile([NCN, w], F16, name=f"yshard{i}")
                        for i, (c0, w) in enumerate(SL)]
            yfull_s = [dp.tile([N, w], F16, addr_space="Shared",
                               name=f"yfull{i}")
                       for i, (c0, w) in enumerate(SL)]

            # ---- node mask * dinv (per node-block) ----
            mdv_sb = sp.tile([P, NTILES], F32)
            with tc.tile_pool(name="wp0", bufs=3) as wp0:
                for k in range(NTILES):
                    rows = min(P, NCN - k * P)
                    xmu = wp0.tile([P, TNF], U8, tag="xmu", bufs=3)
                    nc.sync.dma_start(xmu[:rows], xm[k * P:k * P + rows, :])
                    xmf = wp0.tile([P, TNF], F32, tag="xmf", bufs=3)
                    nc.vector.tensor_copy(out=xmf[:rows], in_=xmu[:rows])
                    mx = wp0.tile([P, 1], F32, tag="mx", bufs=3)
                    nc.vector.reduce_max(out=mx[:rows], in_=xmf[:rows],
                                         axis=mybir.AxisListType.X)
                    nc.vector.tensor_mul(out=mdv_sb[:rows, k:k + 1],
                                         in0=mx[:rows],
                                         in1=dinv_sb[:rows, k:k + 1])

            # ---- hd = z @ W_fc2 + b_fc2 (feature-major: hdT [H, nodes]) ----
            hdT = sp.tile([H, NCN], F16)
            with tc.tile_pool(name="psI", bufs=2, space="PSUM") as psI:
                for off, sz in CH:
                    ph = psI.tile([P, 512], F32, tag="ph", bufs=2)
                    nc.tensor.matmul(out=ph[:, :sz], lhsT=wfc2_sb[:],
                                     rhs=zt_sb[:, off:off + sz],
                                     start=True, stop=True)
                    nc.scalar.activation(
                        out=hdT[:, off:off + sz], in_=ph[:, :sz],
                        func=mybir.ActivationFunctionType.Identity,
                        bias=b2_sb[:, :1])

            # ---- LSTM (T steps, feature-major state) ----
            # Full-width (1250) PSUM per gate; weight loads ordered so each
            # of the 8 weight tiles is loaded once per step.
            cstate = sp.tile([P, NCN], F32)
            nc.vector.memset(cstate[:], 0.0)

            hs = []  # hs[t] tiles [H, NCN]
            hs_pool = tc.tile_pool(name="hspool", bufs=1)
            hsp = hs_pool.__enter__()
            ysb_t = [sp.tile([P, TNF], F16, name=f"ysb_{k}", tag=f"ysb_{k}")
                     for k in range(NTILES)]
            NFULL = NSB - 1  # 78 full source blocks, then a 16-row tail
            ytab = sp.tile([P, NSB * TNF], F16, name="ytab")

            def ship_slice(i):
                c0, w = SL[i]
                for k in range(NTILES):
                    rows = min(P, NCN - k * P)
                    nc.sync.dma_start(yshard_s[i][k * P:k * P + rows, :],
                                      ysb_t[k][:rows, c0:c0 + w])
                nc.gpsimd.collective_compute(
                    "AllGather", mybir.AluOpType.bypass,
                    replica_groups=[list(range(NCORES))],
                    ins=[yshard_s[i].opt()], outs=[yfull_s[i].opt()],
                )
                nc.sync.dma_start(
                    ytab[:, :NFULL * TNF].rearrange(
                        "p (sb f) -> p sb f", f=TNF)[:, :, c0:c0 + w],
                    yfull_s[i][:NFULL * P, :].rearrange(
                        "(sb p) f -> p sb f", p=P))
                nc.sync.dma_start(
                    ytab[:N - NFULL * P,
                         NFULL * TNF + c0:NFULL * TNF + c0 + w],
                    yfull_s[i][NFULL * P:, :])
            with tc.tile_pool(name="psG", bufs=2, space="PSUM") as psG, \
                 tc.tile_pool(name="psY", bufs=2, space="PSUM") as psY, \
                 tc.tile_pool(name="wpL", bufs=2) as wpL:
                def emit_proj(t):
                    for k in range(NTILES):
                        rows = min(P, NCN - k * P)
                        py = psY.tile([P, NF], F32, tag="py", bufs=2)
                        nc.tensor.matmul(out=py[:rows, :],
                                         lhsT=hs[t][:, k * P:k * P + rows],
                                         rhs=wcomb_sb[:],
                                         start=True, stop=True)
                        nc.vector.tensor_scalar(
                            out=ysb_t[k][:rows, t * NF:(t + 1) * NF],
                            in0=py[:rows, :],
                            scalar1=mdv_sb[:rows, k:k + 1],
                            scalar2=None, op0=mybir.AluOpType.mult)
                    if t == 5:
                        ship_slice(0)
                    elif t == 8:
                        ship_slice(1)

                for t in range(T):
                    prev = hdT if t == 0 else hs[t - 1]
                    sg = [None] * 4
                    pqs = [None] * 4

                    def emit_ih(q):
                        wsl = slice(q * H, (q + 1) * H)
                        pqs[q] = psG.tile([P, NCN], F32, name="pq", tag="pq", bufs=2)
                        for off, sz in CH:
                            nc.tensor.matmul(out=pqs[q][:, off:off + sz],
                                             lhsT=wih_sb[:, wsl],
                                             rhs=hdT[:, off:off + sz],
                                             start=True, stop=False)

                    def emit_hh_act(q):
                        wsl = slice(q * H, (q + 1) * H)
                        for off, sz in CH:
                            nc.tensor.matmul(out=pqs[q][:, off:off + sz],
                                             lhsT=whh_sb[:, wsl],
                                             rhs=prev[:, off:off + sz],
                                             start=False, stop=True)
                        sg[q] = wpL.tile([P, NCN], F32, name=f"sg{q}", tag=f"sg{q}", bufs=2)
                        nc.scalar.activation(
                            out=sg[q][:], in_=pqs[q][:],
                            func=getattr(mybir.ActivationFunctionType,
                                         GATE_FUNCS[q]),
                            bias=bg_sb[:, q:q + 1])

                    emit_ih(0)
                    emit_ih(1)
                    if t > 0:
                        emit_proj(t - 1)   # fills PE while h_{t-1} finishes
                    emit_hh_act(0)
                    emit_hh_act(1)
                    for q in (2, 3):
                        emit_ih(q)
                        emit_hh_act(q)

                    nc.vector.tensor_mul(out=cstate[:], in0=cstate[:],
                                         in1=sg[1][:])
                    tmp = wpL.tile([P, NCN], F32, tag="tmp", bufs=2)
                    nc.vector.tensor_mul(out=tmp[:], in0=sg[0][:], in1=sg[2][:])
                    nc.vector.tensor_add(out=cstate[:], in0=cstate[:],
                                         in1=tmp[:])
                    thc = wpL.tile([P, NCN], F32, tag="thc", bufs=2)
                    nc.scalar.activation(
                        out=thc[:], in_=cstate[:],
                        func=mybir.ActivationFunctionType.Tanh)
                    h_t = hsp.tile([P, NCN], F16, name=f"h_{t}", tag=f"h_{t}")
                    nc.vector.tensor_mul(out=h_t[:], in0=sg[3][:], in1=thc[:])
                    hs.append(h_t)
                emit_proj(T - 1)
                ship_slice(2)
            hs_pool.__exit__(None, None, None)  # release hs SBUF before GCN

            # ---- GCN aggregation: agg[k] = sum_sb A[k,sb].T @ Y[sb] ----
            # Whole Y table SBUF-resident: ytab[p, sb*192+f] = Y[sb*128+p, f].
            # A-blocks stream in 8-sb chunks. 2 waves of 5 dst tiles.
            with tc.tile_pool(name="psC", bufs=1, space="PSUM") as psC, \
                 tc.tile_pool(name="wpC", bufs=2) as wpC:
                CHUNK = 8
                sb_chunks = [(s, min(s + CHUNK, NSB))
                             for s in range(0, NSB, CHUNK)]
                for w, wave in enumerate((range(0, 5), range(5, NTILES))):
                    wave = list(wave)
                    pa = {k: psC.tile([P, TNF], F32, name=f"pa_{k}",
                                      tag=f"pa{i}", bufs=1)
                          for i, k in enumerate(wave)}
                    for (s0, s1) in sb_chunks:
                        nsb_c = s1 - s0
                        abc = wpC.tile([P, CHUNK * 5 * P], F16, tag="abc",
                                       bufs=3)
                        r0 = (w * NSB + s0) * P
                        r1 = (w * NSB + s1) * P
                        nc.gpsimd.dma_start(
                            abc[:, :nsb_c * 5 * P].rearrange(
                                "p (sb d) -> p sb d", d=5 * P),
                            ablk[r0:r1, :].rearrange("(sb p) d -> p sb d",
                                                     p=P))
                        for sb in range(s0, s1):
                            srows = min(P, N - sb * P)
                            aoff = (sb - s0) * 5 * P
                            for i, k in enumerate(wave):
                                nc.tensor.matmul(
                                    out=pa[k][:],
                                    lhsT=abc[:srows,
                                             aoff + i * P:aoff + (i + 1) * P],
                                    rhs=ytab[:srows,
                                             sb * TNF:(sb + 1) * TNF],
                                    start=(sb == 0),
                                    stop=(sb == NSB - 1))
                    for i, k in enumerate(wave):
                        rows = min(P, NCN - k * P)
                        osb = wpC.tile([P, TNF], F32, tag="osb", bufs=2)
                        nc.vector.tensor_scalar(out=osb[:rows],
                                                in0=pa[k][:rows],
                                                scalar1=dinv_sb[:rows, k:k + 1],
                                                scalar2=None,
                                                op0=mybir.AluOpType.mult)
                        nc.vector.tensor_add(out=osb[:rows], in0=osb[:rows],
                                             in1=bout_sb[:rows])
                        nc.sync.dma_start(xhat[k * P:k * P + rows, :],
                                          osb[:rows])

    nc.compile()
    return nc


def _preprocess(z, edge_index, x_mask, W_fc2, b_fc2, W_ih, W_hh, b_ih, b_hh,
                W_gcn, b_gcn, W_fc3, b_fc3):
    z = np.asarray(z, np.float32)
    edge_index = np.asarray(edge_index).astype(np.int64)
    x_mask = np.asarray(x_mask)
    src = edge_index[0]
    dst = edge_index[1]
    deg = (np.bincount(dst, minlength=N) + 1.0)
    dinv = (1.0 / np.sqrt(deg)).astype(np.float32)

    src_all = np.concatenate([src, np.arange(N, dtype=np.int64)])
    dst_all = np.concatenate([dst, np.arange(N, dtype=np.int64)])

    # densify adjacency into per-core wave-major A blocks:
    # row (w*NSB + sb)*128 + srel ; col (ktile%5)*128 + drel  (w = ktile//5)
    core_of = dst_all // NCN
    ktile = (dst_all % NCN) // P
    drel = (dst_all % NCN) % P
    sblk = src_all // P
    srel = src_all % P

    a_blocks = []
    lin = (((ktile // 5) * NSB + sblk) * P + srel) * (5 * P) \
        + (ktile % 5) * P + drel
    nblk_lin = 2 * NSB * P * 5 * P
    for c in range(NCORES):
        m = core_of == c
        counts = np.bincount(lin[m], minlength=nblk_lin)
        a_blocks.append(counts.astype(np.float16).reshape(2 * NSB * P, 5 * P))

    Wcomb = np.ascontiguousarray((np.asarray(W_gcn, np.float32)
                                  @ np.asarray(W_fc3, np.float32))
                                 .astype(np.float16))
    bias16 = (np.asarray(b_gcn, np.float32) @ np.asarray(W_fc3, np.float32)
              + np.asarray(b_fc3, np.float32))
    bout_t = np.ascontiguousarray(np.tile(bias16, (P, T)).astype(np.float32))
    bgv = (np.asarray(b_ih, np.float32) + np.asarray(b_hh, np.float32))
    bg_t = np.ascontiguousarray(bgv.reshape(4, P).T.astype(np.float32))
    b2_t = np.ascontiguousarray(np.asarray(b_fc2, np.float32).reshape(P, 1))
    wih_t = np.ascontiguousarray(np.asarray(W_ih, np.float32).T.astype(np.float16))
    whh_t = np.ascontiguousarray(np.asarray(W_hh, np.float32).T.astype(np.float16))
    wfc2_t = np.ascontiguousarray(np.asarray(W_fc2, np.float32))

    in_maps = []
    for c in range(NCORES):
        sl = slice(c * NCN, (c + 1) * NCN)
        dv = dinv[sl]
        dinv_t = np.zeros((P, NTILES), np.float32)
        for k in range(NTILES):
            rows = min(P, NCN - k * P)
            dinv_t[:rows, k] = dv[k * P:k * P + rows]
        in_maps.append({
            "zT": np.ascontiguousarray(z[sl].T),
            "xm": np.ascontiguousarray(
                x_mask[sl].reshape(NCN, TNF).astype(np.uint8)),
            "wfc2": wfc2_t,
            "b2": b2_t,
            "wih": wih_t,
            "whh": whh_t,
            "bg": bg_t,
            "wcomb": Wcomb,
            "bout": bout_t,
            "dinvt": dinv_t,
            "ablk": a_blocks[c],
        })
    return in_maps


def kernel(z, edge_index, x_mask, W_fc2, b_fc2, W_ih, W_hh, b_ih, b_hh,
           W_gcn, b_gcn, W_fc3, b_fc3):
    global LAST_RESULTS
    in_maps = _preprocess(z, edge_index, x_mask, W_fc2, b_fc2,
                          W_ih, W_hh, b_ih, b_hh,
                          W_gcn, b_gcn, W_fc3, b_fc3)
    if "nc" not in _BUILD_CACHE:
        _BUILD_CACHE["nc"] = _build()
    nc = _BUILD_CACHE["nc"]

    trace = bool(int(os.environ.get("KERNEL_TRACE", "0")))
    res = bass_utils.run_bass_kernel_spmd(
        nc, in_maps, core_ids=list(range(NCORES)), trace=trace)
    LAST_RESULTS = res

    out = np.empty((N, T, NF), np.float32)
    for c in range(NCORES):
        out[c * NCN:(c + 1) * NCN] = res.results[c]["xhat"].reshape(NCN, T, NF)
    return out
